# revision 1
# baseline (speedup 1.0000x reference)
"""BinaryConv2d (sign-binarized 3x3 conv, stride 1, pad 1) on 8 Trainium2 cores.

Input  x      [32, 128, 56, 56] f32
       weight [256, 128, 3, 3]  f32  (binarized with sign() before the conv)
       b      [256]             f32
Output        [32, 256, 56, 56] f32

Sharding: data-parallel over the batch dim (4 images per core), binarized
weight replicated to all cores.

Device kernel (per core): conv implemented as 9 shift-matmuls per output
tile accumulating in PSUM. C=128 is the contraction dim (partition dim).
x is pre-padded to [128, 58, 58] so every kernel offset is a pure AP shift
and the image loads as one contiguous DMA per partition. Weights are
pre-binarized/transposed on host to lhsT layout [kh*3+kw, C, O].
Matmuls run in fp16 by default (1 cycle/row, FWL fast weight loads):
binarized weights are exactly +-1 in fp16, so the only precision loss is
rounding x to fp16 (measured 2.1e-4 scale-relative absmax vs float64
reference). BINCONV_DTYPE=f32r selects float32r matmuls (1.0e-4, ~10%
slower); bf16 also supported. A short stream of dummy matmuls with no
data deps runs during the initial DMA wait to flip the PE HAM clock gate
to 2.4 GHz before the real stream starts.
"""

import functools
import os

import numpy as np

# "f32r": fp32 data, float32r matmuls (rel err ~1e-4)
# "fp16": x/weights cast to fp16 (rel err ~2e-4, fast weight loads, half DMA)
# "bf16": x/weights cast to bf16 (rel err ~1.7e-3, same speed as fp16)
DTYPE_MODE = os.environ.get("BINCONV_DTYPE", "fp16")

P = 128          # partitions == input channels per matmul
H = W = 56       # spatial
HP = WP = 58     # padded spatial
O = 256          # output channels
KHW = 9          # 3x3 kernel positions
HT = 8           # output rows per PSUM tile
NT = H // HT     # 7 row tiles
N_CORES = 8
N_PER_CORE = 4   # batch 32 / 8 cores


@functools.lru_cache(maxsize=2)
def _build_nc(mode=DTYPE_MODE):
    import concourse.mybir as mybir
    import concourse.tile as tile
    from concourse import bacc

    mm_dt = {
        "bf16": mybir.dt.bfloat16,
        "fp16": mybir.dt.float16,
        "f32r": mybir.dt.float32r,
    }[mode]
    nc = bacc.Bacc()
    xp = nc.declare_dram_parameter(
        "xp", [N_PER_CORE, P, HP, WP], mm_dt, isOutput=False
    )
    wt = nc.declare_dram_parameter("wt", [KHW, P, O], mm_dt, isOutput=False)
    bias = nc.declare_dram_parameter("bias", [O], mybir.dt.float32, isOutput=False)
    out = nc.declare_dram_parameter(
        "out", [N_PER_CORE, O, H, W], mybir.dt.float32, isOutput=True
    )
    xp_ap = xp[:]
    wt_ap = wt[:]
    bias_ap = bias[:]
    out_ap = out[:]

    with tile.TileContext(nc) as tc:
        with (
            tc.tile_pool(name="wpool", bufs=1) as wpool,
            tc.tile_pool(name="xpool", bufs=8) as xpool,
            tc.tile_pool(name="opool", bufs=4) as opool,
            tc.tile_pool(name="psum", bufs=4, space="PSUM") as pp,
        ):
            # Weight/bias triggers go on the scalar (ACT) queue so
            # they don't serialize behind x-chunk triggers on sync. Split by
            # o-half: the first matmul group only needs the oh=0 half.
            wt_sb = wpool.tile([P, KHW, O], mm_dt)
            wt_t = wt_ap.rearrange("k c o -> c k o")
            nc.scalar.dma_start(wt_sb[:, :, 0:P], wt_t[:, :, 0:P])
            nc.scalar.dma_start(wt_sb[:, :, P:O], wt_t[:, :, P:O])
            b_sb = wpool.tile([P, 2], mybir.dt.float32)
            nc.scalar.dma_start(b_sb[:], bias_ap.rearrange("(g p) -> p g", p=P))

            # PE warmup: ~5us of dummy matmuls with no data deps. They run
            # during the initial DMA wait and flip the HAM clock gate to
            # 2.4 GHz before the real matmul stream begins (saves the
            # ~2us cold-ramp and the first groups run warm).
            warm_sb = wpool.tile([P, HT * W], mm_dt)
            nc.gpsimd.memset(warm_sb[:], 0.0)
            warm_ps = pp.tile([P, 2, 512], mybir.dt.float32, tag="pt")
            N_WARM = 16
            for i in range(N_WARM):
                nc.tensor.matmul(
                    warm_ps[:, 0, 0 : HT * W],
                    warm_sb[:, 0:P],
                    warm_sb[:],
                    start=(i == 0),
                    stop=(i == N_WARM - 1),
                )

            # Row-tiles are processed in pairs sharing one 2-bank PSUM tile,
            # so one ACT eviction + one output DMA covers 16 rows. Fewer
            # cross-engine syncs = fewer semaphores = shorter exit-drain
            # sem-reset storm (~7us of the tail scales with sem count).
            NF = HT * W  # 448 matmul free size
            for n in range(N_PER_CORE):
                # x loads as 4 halo chunks (18/18/18/10 rows): chunk c serves
                # row-tile pair (2c, 2c+1); the first matmul group only waits
                # on 0.5 MB, not the whole image.
                chunks = []
                for c in range(4):
                    r0 = 16 * c
                    rows = min(18, HP - r0)
                    xc = xpool.tile([P, 18, WP], mm_dt, tag="xc")
                    nc.sync.dma_start(
                        xc[:, 0:rows, :], xp_ap[n, :, r0 : r0 + rows, :]
                    )
                    chunks.append(xc)
                for oh in range(2):
                    osl = slice(oh * P, (oh + 1) * P)
                    for i in range(4):
                        ts_pair = [t for t in (2 * i, 2 * i + 1) if t < NT]
                        pt = pp.tile([P, 2, 512], mybir.dt.float32, tag="pt")
                        for j, t in enumerate(ts_pair):
                            x_sb = chunks[t // 2]
                            loc = HT * (t - 2 * (t // 2))
                            for kh in range(3):
                                for kw in range(3):
                                    kk = kh * 3 + kw
                                    nc.tensor.matmul(
                                        pt[:, j, 0:NF],
                                        wt_sb[:, kk, osl],
                                        x_sb[:, loc + kh : loc + kh + HT, kw : kw + W],
                                        start=(kk == 0),
                                        stop=(kk == KHW - 1),
                                    )
                        npair = len(ts_pair)
                        ot = opool.tile([P, 2, HT, W], mybir.dt.float32)
                        nc.scalar.add(
                            ot[:, 0:npair],
                            pt[:, 0:npair, 0:NF].rearrange(
                                "p a (h w) -> p a h w", h=HT
                            ),
                            b_sb[:, oh : oh + 1],
                        )
                        r0 = HT * ts_pair[0]
                        r1 = HT * (ts_pair[-1] + 1)
                        nc.sync.dma_start(
                            out_ap[n, osl, r0:r1, :].rearrange(
                                "o (a h) w -> o a h w", h=HT
                            ),
                            ot[:, 0:npair],
                        )
    nc.finalize()
    return nc


def _prep(x, weight, b, mode=DTYPE_MODE):
    x = np.asarray(x, dtype=np.float32)
    w = np.asarray(weight, dtype=np.float32)
    b = np.ascontiguousarray(np.asarray(b, dtype=np.float32))
    bw = np.sign(w)  # matches torch/jax sign: sign(0) = 0
    # [O, C, kh, kw] -> [kh*3+kw, C, O] (lhsT layout: contraction on partitions)
    wt = np.ascontiguousarray(bw.transpose(2, 3, 1, 0).reshape(KHW, P, O))
    np_dt = np.float32
    if mode == "bf16":
        import ml_dtypes

        np_dt = ml_dtypes.bfloat16
    elif mode == "fp16":
        np_dt = np.float16
    if np_dt is not np.float32:
        wt = wt.astype(np_dt)  # +-1/0 exact in bf16/fp16
    xp = np.zeros((x.shape[0], P, HP, WP), np_dt)
    xp[:, :, 1 : H + 1, 1 : W + 1] = x.astype(np_dt)
    return xp, wt, b


def _run(in_maps, trace=False):
    from concourse.bass_utils import run_bass_kernel_spmd

    nc = _build_nc()
    return run_bass_kernel_spmd(
        nc, in_maps, core_ids=list(range(N_CORES)), trace=trace
    )


def kernel(x, weight, b):
    xp, wt, bias = _prep(x, weight, b)
    in_maps = [
        {
            "xp": np.ascontiguousarray(xp[c * N_PER_CORE : (c + 1) * N_PER_CORE]),
            "wt": wt,
            "bias": bias,
        }
        for c in range(N_CORES)
    ]
    res = _run(in_maps, trace=False)
    return np.concatenate([r["out"] for r in res.results], axis=0)

